# revision 1
# baseline (speedup 1.0000x reference)
"""Birman-Schwinger core: K[b] = diag(sqrt|V_b|) @ R_0 @ diag(sqrt|V_b|).

Key identity: with g[b,u] = sqrt(|V[b,u]| + eps) / (1 + u) and d = u - v,

    K[b,u,v] = g[b,u] * g[b,v] * H(d)
    H(d) = 0.5j * exp(2j*d) * sign(d)
         = -0.5*sign(d)*sin(2d)  +  0.5j*sign(d)*cos(2d)

H is a fixed Toeplitz table, so each 128-row block of the output is an
elementwise product of a sliding window of the (host-precomputed) H table,
a per-partition scalar g_u, and a broadcast row g_v — one fused
scalar_tensor_tensor op on the Vector engine per tile, then DMA out.

Sharding: 8 cores; core c handles batch b = c // 2, row half h = c % 2
(rows [2048*h, 2048*h + 2048) of the (4096, 4096) complex output).
Output is written as interleaved re/im f32 pairs so the per-core
(2048, 8192) f32 result is exactly the complex64 memory layout.
"""

import numpy as np

B = 4
N = 4096
NCORES = 8
HALF = N // 2            # rows per core
P = 128                  # SBUF partitions
NBLK = HALF // P         # 16 row blocks per core
EPS = 1e-10
MW = 4096 + 15 * P       # master table width in complex columns (6016)
SHIFT = 15 * P           # block k slice starts at complex col SHIFT - P*k
CHUNK = 4096             # f32 columns per compute/DMA tile (2048 complex)
NCHUNK = (2 * N) // CHUNK

_PROGRAM_CACHE = {}


def _build_program():
    import concourse.bacc as bacc
    import concourse.mybir as mybir
    from concourse.tile import TileContext

    nc = bacc.Bacc("TRN2", target_bir_lowering=False, debug=False)
    m = nc.dram_tensor("t_m", [P, 2 * MW], mybir.dt.float32, kind="ExternalInput").ap()
    g3 = nc.dram_tensor(
        "t_g3", [3, 2 * N], mybir.dt.bfloat16, kind="ExternalInput"
    ).ap()
    ones = nc.dram_tensor("t_ones", [3, P], mybir.dt.bfloat16, kind="ExternalInput").ap()
    gu = nc.dram_tensor("t_gu", [P, NBLK], mybir.dt.float32, kind="ExternalInput").ap()
    out = nc.dram_tensor(
        "t_out", [HALF, 2 * N], mybir.dt.float32, kind="ExternalOutput"
    ).ap()
    mult = mybir.AluOpType.mult

    with TileContext(nc) as tc:
        with tc.tile_pool(name="const", bufs=1) as cpool:
            m_sb = cpool.tile([P, 2 * MW], mybir.dt.float32)
            gvb_sb = cpool.tile([P, 2 * N], mybir.dt.float32)
            gu_sb = cpool.tile([P, NBLK], mybir.dt.float32)
            # Broadcast staging lives in a nested pool so its SBUF range is
            # released to the work pool once the broadcast is emitted.
            with (
                tc.tile_pool(name="bcast", bufs=1) as bpool,
                tc.tile_pool(name="psum", bufs=8, space="PSUM") as ppool,
            ):
                g3_sb = bpool.tile([3, 2 * N], mybir.dt.bfloat16)
                ones_sb = bpool.tile([3, P], mybir.dt.bfloat16)
                nc.sync.dma_start(out=ones_sb[:, :], in_=ones[:, :])
                nc.sync.dma_start(out=g3_sb[:, :], in_=g3[:, :])
                nc.sync.dma_start(out=gu_sb[:, :], in_=gu[:, :])
                # g_v broadcast: ones[3,128]^T @ [g_hi; g_mid; g_lo] on the
                # (idle) Tensor engine. The bf16 triple-split sums back to g
                # within ~1 ulp in the fp32 PSUM accumulator, and bf16 matmul
                # is single-pass (fast). ScalarE drains each PSUM bank into
                # the SBUF broadcast tile. Replaces a 4MB HBM load.
                BCW = 512
                for q in range(0, 2 * N, BCW):
                    pt = ppool.tile([P, BCW], mybir.dt.float32)
                    nc.tensor.matmul(
                        out=pt[:, :],
                        lhsT=ones_sb[:, :],
                        rhs=g3_sb[:, q : q + BCW],
                        start=True,
                        stop=True,
                    )
                    nc.scalar.copy(out=gvb_sb[:, q : q + BCW], in_=pt[:, :])
                # Load the master table in consumption order (descending-k
                # blocks read columns low to high); the first compute chunk
                # only needs cols [0, CHUNK).
                for q0 in range(0, 2 * MW, CHUNK):
                    q1 = min(q0 + CHUNK, 2 * MW)
                    nc.sync.dma_start(out=m_sb[:, q0:q1], in_=m[:, q0:q1])

            wpool_cm = tc.tile_pool(name="work", bufs=7)
            wpool = wpool_cm.__enter__()
            ci = 0
            for k in reversed(range(NBLK)):
                s2 = 2 * (SHIFT - P * k)  # f32 col offset of block k's window
                # Smaller tiles for the first block (starts sooner behind
                # the broadcast) and the last blocks (finer store-backlog
                # drain at the end).
                cw = CHUNK if 3 <= k < NBLK - 1 else CHUNK // 2
                for c0 in range(0, 2 * N, cw):
                    t = wpool.tile([P, cw], mybir.dt.float32)
                    nc.vector.scalar_tensor_tensor(
                        out=t[:, :],
                        in0=m_sb[:, s2 + c0 : s2 + c0 + cw],
                        scalar=gu_sb[:, k : k + 1],
                        in1=gvb_sb[:, c0 : c0 + cw],
                        op0=mult,
                        op1=mult,
                    )
                    # Alternate output DMAs across the two HWDGE rings
                    # (SP and ACT) for more in-flight descriptors.
                    dma_eng = nc.sync if ci % 2 == 0 else nc.scalar
                    dma_eng.dma_start(
                        out=out[k * P : (k + 1) * P, c0 : c0 + cw],
                        in_=t[:, :],
                    )
                    ci += 1
            wpool_cm.__exit__(None, None, None)
    nc.compile()
    return nc


def _get_program():
    if "nc" not in _PROGRAM_CACHE:
        _PROGRAM_CACHE["nc"] = _build_program()
    return _PROGRAM_CACHE["nc"]


def _host_tables(V):
    """Per-core input arrays (all f32)."""
    pos = np.arange(N, dtype=np.float64)
    g = (np.sqrt(np.abs(V).astype(np.float64) + EPS) / (1.0 + pos)).astype(
        np.float32
    )  # (B, N)

    masters = {}
    for h in range(2):
        u0 = HALF * h
        p = np.arange(P, dtype=np.int64)[:, None]
        j = np.arange(MW, dtype=np.int64)[None, :]
        d = p - j + u0 + SHIFT
        s = np.sign(d).astype(np.float64)
        hre = -0.5 * s * np.sin(2.0 * d)
        him = 0.5 * s * np.cos(2.0 * d)
        inter = np.empty((P, 2 * MW), dtype=np.float32)
        inter[:, 0::2] = hre
        inter[:, 1::2] = him
        masters[h] = inter

    in_maps = []
    for c in range(NCORES):
        b, h = divmod(c, 2)
        u0 = HALF * h
        grow = np.empty(2 * N, dtype=np.float32)
        grow[0::2] = g[b]
        grow[1::2] = g[b]
        import ml_dtypes

        bf16 = ml_dtypes.bfloat16
        g3 = np.empty((3, 2 * N), dtype=bf16)
        g3[0] = grow.astype(bf16)
        r1 = grow - g3[0].astype(np.float32)
        g3[1] = r1.astype(bf16)
        r2 = r1 - g3[1].astype(np.float32)
        g3[2] = r2.astype(bf16)
        gu = np.ascontiguousarray(g[b, u0 : u0 + HALF].reshape(NBLK, P).T)
        ones = np.ones((3, P), dtype=bf16)
        in_maps.append({"t_m": masters[h], "t_g3": g3, "t_gu": gu, "t_ones": ones})
    return in_maps


def _run(in_maps, trace=False, **kwargs):
    from concourse import bass_utils

    nc = _get_program()
    return bass_utils.run_bass_kernel_spmd(
        nc, in_maps, core_ids=list(range(NCORES)), trace=trace, **kwargs
    )


def kernel(V):
    V = np.asarray(V, dtype=np.float32)
    assert V.shape == (B, N), V.shape
    in_maps = _host_tables(V)
    res = _run(in_maps, trace=False)
    out = np.empty((B, N, N), dtype=np.complex64)
    for c in range(NCORES):
        b, h = divmod(c, 2)
        plane = np.ascontiguousarray(res.results[c]["t_out"])
        out[b, HALF * h : HALF * (h + 1), :] = plane.view(np.complex64)
    return out



# revision 2
# speedup vs baseline: 2.2734x; 2.2734x over previous
"""Birman-Schwinger core: K[b] = diag(sqrt|V_b|) @ R_0 @ diag(sqrt|V_b|).

Key identities: with g[b,u] = sqrt(|V[b,u]| + eps) / (1 + u) and d = u - v,

    K[b,u,v] = g[b,u] * g[b,v] * H(d),   H(d) = 0.5j * exp(2j*d) * sign(d)

and K is Hermitian (K[b,v,u] = conj(K[b,u,v]), zero diagonal), so the
device only computes a lower-triangular trapezoid cover of each (N, N)
plane in fp16; the host mirrors the strict upper triangle via conj().
This halves both the vector work and (with fp16) quarters the HBM write
traffic vs the full-complex64 plane.

Sharding: 8 cores; core c handles batch b = c // 2.  The 32 row-blocks
(128 rows each) of a plane are split by parity: even cores take odd
blocks k = 1,3,..,31, odd cores take even blocks k = 0,2,..,30 padded to
the same widths, so one compiled program serves all cores.  Local block
i (0..15) maps to k = 2i+1 (resp. 2i) and writes columns [0, 256(i+1))
of rows [128k, 128k+128) -- identical tile shapes and identical SBUF
slice offsets on every core; only the DRAM input payloads differ.

H is a Toeplitz table: a (128, 4096)-complex master with
master[p, m] = H(p - m + C) (C = 3968 even cores / 3840 odd) serves all
16 blocks as a sliding window at fp16-col offset 7680 - 512i.  Each
output tile is one fused scalar_tensor_tensor on the Vector engine
(fp16 in/out, eligible for the 2x DVE perf mode), then DMA'd out.
"""

import numpy as np

B = 4
N = 4096
NCORES = 8
P = 128
NB = 16                  # local row blocks per core
MW = 4096                # master width in complex columns
OUTW = 2 * N             # out row width in fp16 elements (8192)
EPS = 1e-10
MAXCW = 4096             # max fp16 cols per compute/DMA tile

_PROGRAM_CACHE = {}


def _build_program():
    import concourse.bacc as bacc
    import concourse.mybir as mybir
    from concourse.tile import TileContext

    nc = bacc.Bacc("TRN2", target_bir_lowering=False, debug=False)
    m = nc.dram_tensor("t_m", [P, 2 * MW], mybir.dt.float16, kind="ExternalInput").ap()
    g3 = nc.dram_tensor("t_g3", [2, OUTW], mybir.dt.bfloat16, kind="ExternalInput").ap()
    ones = nc.dram_tensor("t_ones", [2, P], mybir.dt.bfloat16, kind="ExternalInput").ap()
    gu = nc.dram_tensor("t_gu", [P, NB], mybir.dt.float16, kind="ExternalInput").ap()
    out = nc.dram_tensor(
        "t_out", [NB * P, OUTW], mybir.dt.float16, kind="ExternalOutput"
    ).ap()
    mult = mybir.AluOpType.mult

    with TileContext(nc) as tc:
        with tc.tile_pool(name="const", bufs=1) as cpool:
            m_sb = cpool.tile([P, 2 * MW], mybir.dt.float16)
            gvb_sb = cpool.tile([P, OUTW], mybir.dt.float16)
            gu_sb = cpool.tile([P, NB], mybir.dt.float16)
            with (
                tc.tile_pool(name="bcast", bufs=1) as bpool,
                tc.tile_pool(name="psum", bufs=8, space="PSUM") as ppool,
            ):
                g3_sb = bpool.tile([2, OUTW], mybir.dt.bfloat16)
                ones_sb = bpool.tile([2, P], mybir.dt.bfloat16)
                nc.sync.dma_start(out=ones_sb[:, :], in_=ones[:, :])
                nc.sync.dma_start(out=g3_sb[:, :], in_=g3[:, :])
                nc.sync.dma_start(out=gu_sb[:, :], in_=gu[:, :])
                # g_v broadcast: ones[2,128]^T @ [g_hi; g_lo] on the idle
                # Tensor engine; the bf16 2-term split reconstructs g to
                # ~fp24 in the f32 PSUM accumulator, then ScalarE drains
                # each PSUM bank into the fp16 broadcast row tile.
                BCW = 512
                for q in range(0, OUTW, BCW):
                    pt = ppool.tile([P, BCW], mybir.dt.float32)
                    nc.tensor.matmul(
                        out=pt[:, :],
                        lhsT=ones_sb[:, :],
                        rhs=g3_sb[:, q : q + BCW],
                        start=True,
                        stop=True,
                    )
                    nc.scalar.copy(out=gvb_sb[:, q : q + BCW], in_=pt[:, :])
                # Master load in consumption order: block i's window starts
                # at fp16 col 7680 - 512i, so early blocks need the tail.
                MCH = 2048
                for q1 in range(2 * MW, 0, -MCH):
                    nc.sync.dma_start(out=m_sb[:, q1 - MCH : q1], in_=m[:, q1 - MCH : q1])

            wpool_cm = tc.tile_pool(name="work", bufs=6)
            wpool = wpool_cm.__enter__()
            ci = 0
            for i in range(NB):
                w = 512 * (i + 1)        # fp16 cols written by this block
                c0 = 2 * MW - 512 - 512 * i  # master fp16 col offset
                nchunks = 1 if w <= MAXCW else 2
                cw = w // nchunks
                for j in range(nchunks):
                    t = wpool.tile([P, cw], mybir.dt.float16)
                    nc.vector.scalar_tensor_tensor(
                        out=t[:, :],
                        in0=m_sb[:, c0 + j * cw : c0 + (j + 1) * cw],
                        scalar=gu_sb[:, i : i + 1],
                        in1=gvb_sb[:, j * cw : (j + 1) * cw],
                        op0=mult,
                        op1=mult,
                    )
                    # Alternate output DMAs across the two HWDGE rings.
                    dma_eng = nc.sync if ci % 2 == 0 else nc.scalar
                    dma_eng.dma_start(
                        out=out[i * P : (i + 1) * P, j * cw : (j + 1) * cw],
                        in_=t[:, :],
                    )
                    ci += 1
            wpool_cm.__exit__(None, None, None)
    nc.compile()
    return nc


def _get_program():
    if "nc" not in _PROGRAM_CACHE:
        _PROGRAM_CACHE["nc"] = _build_program()
    return _PROGRAM_CACHE["nc"]


def _host_tables(V):
    """Per-core input arrays."""
    import ml_dtypes

    bf16 = ml_dtypes.bfloat16
    pos = np.arange(N, dtype=np.float64)
    g = (np.sqrt(np.abs(V).astype(np.float64) + EPS) / (1.0 + pos)).astype(
        np.float32
    )  # (B, N)

    # Masters per core parity t: C = 3968 (t=0, odd blocks) / 3840 (t=1).
    masters = {}
    p = np.arange(P, dtype=np.int64)[:, None]
    mcol = np.arange(MW, dtype=np.int64)[None, :]
    for t, C in ((0, 3968), (1, 3840)):
        d = (p - mcol + C).astype(np.float64)
        s = np.sign(d)
        inter = np.empty((P, 2 * MW), dtype=np.float16)
        inter[:, 0::2] = -0.5 * s * np.sin(2.0 * d)
        inter[:, 1::2] = 0.5 * s * np.cos(2.0 * d)
        masters[t] = inter

    in_maps = []
    ones = np.ones((2, P), dtype=bf16)
    for c in range(NCORES):
        b, t = divmod(c, 2)
        grow = np.empty(OUTW, dtype=np.float32)
        grow[0::2] = g[b]
        grow[1::2] = g[b]
        g3 = np.empty((2, OUTW), dtype=bf16)
        g3[0] = grow.astype(bf16)
        g3[1] = (grow - g3[0].astype(np.float32)).astype(bf16)
        gut = np.empty((P, NB), dtype=np.float16)
        for i in range(NB):
            k = 2 * i + 1 if t == 0 else 2 * i
            gut[:, i] = g[b, 128 * k : 128 * k + P]
        in_maps.append({"t_m": masters[t], "t_g3": g3, "t_gu": gut, "t_ones": ones})
    return in_maps


def _run(in_maps, trace=False, **kwargs):
    from concourse import bass_utils

    nc = _get_program()
    return bass_utils.run_bass_kernel_spmd(
        nc, in_maps, core_ids=list(range(NCORES)), trace=trace, **kwargs
    )


def kernel(V):
    V = np.asarray(V, dtype=np.float32)
    assert V.shape == (B, N), V.shape
    in_maps = _host_tables(V)
    res = _run(in_maps, trace=False)

    cols = np.arange(N)[None, :]
    kk = np.arange(N)[:, None] // P
    W = np.where(kk % 2 == 1, P * (kk + 1), P * (kk + 2))
    mask = cols < W  # device-written region per row

    out = np.empty((B, N, N), dtype=np.complex64)
    for b in range(B):
        plane = np.empty((N, N), dtype=np.complex64)
        for t in (0, 1):
            r = np.asarray(res.results[2 * b + t]["t_out"])
            for i in range(NB):
                k = 2 * i + 1 if t == 0 else 2 * i
                w = 512 * (i + 1)
                tile = np.ascontiguousarray(r[i * P : (i + 1) * P, :w])
                plane[128 * k : 128 * k + P, : w // 2] = tile.astype(
                    np.float32
                ).view(np.complex64)
        out[b] = np.where(mask, plane, np.conj(plane.T))
    return out
